# revision 6
# baseline (speedup 1.0000x reference)
"""CLUB loss kernel for Trainium2 (8 NeuronCores, SPMD row-sharded).

Math: the reference returns mean_i(pos_i - neg_i), a scalar:

  mean_pos = -0.5/N * (A - 2B + C)
      A = sum_{i,d} x^2 * invv,  B = sum x*mu*invv,  C = sum mu^2*invv
  mean_neg = -0.5/N^2 * (S_invv . S_x2 - 2*S_muinvv . S_x + N*C)
  loss = mean_pos - mean_neg

C cancels exactly in the loss, so we never compute it.  The host also
pre-scales mu' = -2*mu, which folds the -2B into a single fused sum:

  P := A - 2B = sum([x | x^2] * [mu'*invv | invv])   (one STT pass)
  loss = -0.5/N * P + 0.5/N^2 * (S_invv.S_x2 + S_mi'.S_x)
      where S_mi' = sum mu'*invv = -2*S_muinvv

Each core handles 2048 rows; layout is d-major (128, 1024): partition
q = (sub-slab b, dim d), free axis = row index, so every reduction is a
free-axis row-sum fused into the producing instruction via accum_out.

Per column-chunk h the SBUF arena tile is laid out
  [ lv | mu' | x | x2 | mi' | invv ]   (6*C cols)
so the DMA lands [lv|mu'|x] as ONE contiguous dma_start (128 descriptors
of 3*C*4 bytes - large packets keep the HWDGE queues at full rate), and
the P pass reads the adjacent [x|x2] and [mi'|invv] spans as single APs.

Compute per chunk (engines balanced; walrus rejects gpsimd STT/TS and
accum, so PL only does its TensorTensor product):
  ACT: invv = exp(-lv) (+Sinvv), x2 = x^2 (+Sx2), Sm copy-acc (chunk 0)
  PL : mi' = mu' * invv
  DVE: P fused STT (+acc), Sx TS (+acc), Sm TS (+acc, chunks 1-2)

Output: acc [128, 15] f32, DMA'd out as two partition-halves on the two
HWDGE queues in parallel; the host combines all cores in float64.
"""

import sys

sys.path.insert(0, "/opt/trn_rl_repo")

import numpy as np
from contextlib import ExitStack

import concourse.bass as bass
import concourse.bacc as bacc
import concourse.tile as tile
from concourse import mybir
from concourse.bass_utils import run_bass_kernel_spmd

F32 = mybir.dt.float32
N_CORES = 8
B, D, H, W = 16, 64, 32, 32
HW = H * W                # 1024
N = B * HW                # 16384
NB = B // N_CORES         # 2 sub-slabs (batches) per core
ROWS = NB * HW            # 2048 rows per core
COLS = HW                 # free size of the (128, 1024) layout
QUANT = ["P", "Sx", "Sm", "Sx2", "Sinvv"]
BOUNDS = [0, 320, 832, 1024]
NCH = len(BOUNDS) - 1
CS = [BOUNDS[h + 1] - BOUNDS[h] for h in range(NCH)]
# which chunks' Sm pass runs on ACT (copy+acc) vs DVE (TS+acc)
SM_ON_ACT = set()


def build_nc() -> bass.Bass:
    nc = bacc.Bacc()
    ins = [
        nc.dram_tensor(f"in{h}", [128, 3 * CS[h]], F32, kind="ExternalInput")
        for h in range(NCH)
    ]
    accs = nc.dram_tensor("accs", [128, len(QUANT) * NCH], F32,
                          kind="ExternalOutput")

    with ExitStack() as ctx:
        tc = ctx.enter_context(tile.TileContext(nc))
        big = ctx.enter_context(tc.tile_pool(name="big", bufs=1))
        jp = ctx.enter_context(tc.tile_pool(name="jp", bufs=2))
        accp = ctx.enter_context(tc.tile_pool(name="accp", bufs=1))

        # per-chunk arena: [lv | mu' | x | x2 | mi' | invv]
        ar = [big.tile([128, 6 * CS[h]], F32, name=f"ar{h}")
              for h in range(NCH)]
        acc = accp.tile([128, len(QUANT) * NCH], F32)

        def col(q, c):
            i = QUANT.index(q) * NCH + c
            return acc[:, i:i + 1]

        # ch0 (small) + ch2 on the ACT queue (both gens issued before any
        # ACT compute, which only starts once ch0 lands anyway); ch1 on SP.
        # No early ACT op may precede these gens - in particular no memzero,
        # whose activation would pull the act-table load ahead of the gens.
        qs = {0: nc.scalar, 1: nc.sync, 2: nc.scalar}
        for h in (0, 2, 1):
            C = CS[h]
            qs[h].dma_start(out=ar[h][:, 0:3 * C], in_=ins[h][:, :])

        M = mybir.AluOpType.mult
        ADD = mybir.AluOpType.add
        EXP = mybir.ActivationFunctionType.Exp
        SQ = mybir.ActivationFunctionType.Square
        CP = mybir.ActivationFunctionType.Copy

        for h in range(NCH):
            C = CS[h]
            lv_s = ar[h][:, 0:C]
            mu_s = ar[h][:, C:2 * C]
            x_s = ar[h][:, 2 * C:3 * C]
            x2_s = ar[h][:, 3 * C:4 * C]
            mi_s = ar[h][:, 4 * C:5 * C]
            iv_s = ar[h][:, 5 * C:6 * C]
            xx2_s = ar[h][:, 2 * C:4 * C]   # [x | x2]
            miiv_s = ar[h][:, 4 * C:6 * C]  # [mi' | invv]

            nc.scalar.activation(
                out=iv_s, in_=lv_s, func=EXP, bias=0.0, scale=-1.0,
                accum_out=col("Sinvv", h),
            )
            nc.scalar.activation(
                out=x2_s, in_=x_s, func=SQ, bias=0.0, scale=1.0,
                accum_out=col("Sx2", h),
            )
            nc.gpsimd.tensor_tensor(mi_s, mu_s, iv_s, op=M)

            # P = sum x*mi' + x2*invv = A - 2B, one fused pass
            jp_t = jp.tile([128, 2 * C], F32, tag="jp", name=f"jp{h}")
            nc.vector.scalar_tensor_tensor(
                out=jp_t[:], in0=xx2_s, scalar=1.0, in1=miiv_s,
                op0=M, op1=M, accum_out=col("P", h),
            )
            jx = jp.tile([128, C], F32, tag="jx", name=f"jx{h}")
            nc.vector.tensor_scalar(
                out=jx[:], in0=x_s, scalar1=1.0, scalar2=0.0,
                op0=M, op1=ADD, accum_out=col("Sx", h),
            )
            if h in SM_ON_ACT:
                jm = jp.tile([128, C], F32, tag="jm", name=f"jm{h}")
                nc.scalar.activation(
                    out=jm[:], in_=mi_s, func=CP, bias=0.0, scale=1.0,
                    accum_out=col("Sm", h),
                )
            else:
                jm = jp.tile([128, C], F32, tag="jm", name=f"jm{h}")
                nc.vector.tensor_scalar(
                    out=jm[:], in0=mi_s, scalar1=1.0, scalar2=0.0,
                    op0=M, op1=ADD, accum_out=col("Sm", h),
                )

        nc.sync.dma_start(out=accs[0:64, :], in_=acc[0:64, :])
        nc.scalar.dma_start(out=accs[64:128, :], in_=acc[64:128, :])
    return nc


def _ensure_ntff_hook():
    """This image's antenv lacks axon_hooks; if tracing is requested
    (e.g. BASS_TRACE=1), run_bass_kernel_spmd would die on the import.
    Register the ctypes-based hook if available, else a None hook so
    tracing degrades gracefully."""
    import types

    if "antenv.axon_hooks" in sys.modules:
        return
    try:
        import antenv.axon_hooks  # noqa: F401
        return
    except ImportError:
        pass
    hook = None
    try:
        sys.path.insert(0, "/root/.axon_site")
        from trn_agent_boot.trn_boot import _ntff_profile_via_ctypes

        hook = _ntff_profile_via_ctypes("/opt/axon/libaxon_pjrt.so")
    except Exception:
        hook = None
    mod = types.ModuleType("antenv.axon_hooks")
    mod._hook = hook
    mod.get_axon_ntff_profile_hook = lambda: mod._hook
    mod.set_axon_ntff_profile_hook = lambda h: setattr(mod, "_hook", h)
    sys.modules["antenv.axon_hooks"] = mod


_ensure_ntff_hook()

_NC = None


def _get_nc():
    global _NC
    if _NC is None:
        _NC = build_nc()
        # bacc passes legalize multi-sync-wait instructions for TRN2 codegen
        _NC.compile()
    return _NC


def make_in_maps(x, mu, logvar):
    x = np.ascontiguousarray(np.asarray(x, dtype=np.float32))
    mu = np.asarray(mu, dtype=np.float32)
    lv = np.asarray(logvar, dtype=np.float32)
    in_maps = []
    for c in range(N_CORES):
        r0 = c * ROWS
        mu_t = np.concatenate(
            [mu[r0 + b * HW:r0 + (b + 1) * HW].T for b in range(NB)], axis=0
        ) * np.float32(-2.0)
        lv_t = np.concatenate(
            [lv[r0 + b * HW:r0 + (b + 1) * HW].T for b in range(NB)], axis=0
        )
        x_t = x[c * NB:(c + 1) * NB].reshape(128, COLS)
        m = {}
        for h in range(NCH):
            sl = slice(BOUNDS[h], BOUNDS[h + 1])
            m[f"in{h}"] = np.ascontiguousarray(
                np.concatenate([lv_t[:, sl], mu_t[:, sl], x_t[:, sl]], axis=1)
            )
        in_maps.append(m)
    return in_maps


def combine(results) -> np.ndarray:
    nq = len(QUANT)
    tot = np.zeros((nq, 128), dtype=np.float64)
    for r in results:
        a = np.asarray(r["accs"], dtype=np.float64)  # (128, nq*NCH)
        for q in range(nq):
            tot[q] += a[:, q * NCH:(q + 1) * NCH].sum(axis=1)
    P = tot[QUANT.index("P")].sum()
    vec = {q: tot[QUANT.index(q)].reshape(NB, D).sum(axis=0)
           for q in ("Sx", "Sm", "Sx2", "Sinvv")}
    # Sm is -2*S_muinvv, so  S_invv.S_x2 - 2*S_muinvv.S_x  =  Si.Sx2 + Sm.Sx
    loss = (-0.5 / N * P
            + 0.5 / float(N) ** 2 * (vec["Sinvv"] @ vec["Sx2"]
                                     + vec["Sm"] @ vec["Sx"]))
    return np.array(loss, dtype=np.float32)


def kernel(x, mu, logvar, **_kwargs):
    nc = _get_nc()
    in_maps = make_in_maps(x, mu, logvar)
    res = run_bass_kernel_spmd(nc, in_maps, list(range(N_CORES)))
    return combine(res.results)


# revision 9
# speedup vs baseline: 1.0072x; 1.0072x over previous
"""CLUB loss kernel for Trainium2 (8 NeuronCores, SPMD row-sharded).

Math: the reference returns mean_i(pos_i - neg_i), a scalar:

  mean_pos = -0.5/N * (A - 2B + C)
      A = sum_{i,d} x^2 * invv,  B = sum x*mu*invv,  C = sum mu^2*invv
  mean_neg = -0.5/N^2 * (S_invv . S_x2 - 2*S_muinvv . S_x + N*C)
  loss = mean_pos - mean_neg

C cancels exactly in the loss, so we never compute it.  The host also
pre-scales mu' = -2*mu, which folds the -2B into a single fused sum:

  P := A - 2B = sum([x | x^2] * [mu'*invv | invv])   (one STT pass)
  loss = -0.5/N * P + 0.5/N^2 * (S_invv.S_x2 + S_mi'.S_x)
      where S_mi' = sum mu'*invv = -2*S_muinvv

Each core handles 2048 rows; layout is d-major (128, 1024): partition
q = (sub-slab b, dim d), free axis = row index, so every reduction is a
free-axis row-sum fused into the producing instruction via accum_out.

Per column-chunk h the SBUF arena tile is laid out
  [ lv | mu' | x | x2 | mi' | invv ]   (6*C cols)
so the DMA lands [lv|mu'|x] as ONE contiguous dma_start (128 descriptors
of 3*C*4 bytes - large packets keep the HWDGE queues at full rate), and
the P pass reads the adjacent [x|x2] and [mi'|invv] spans as single APs.

Compute per chunk (engines balanced; walrus rejects gpsimd STT/TS and
accum, so PL only does its TensorTensor product):
  ACT: invv = exp(-lv) (+Sinvv), x2 = x^2 (+Sx2), Sm copy-acc (chunk 0)
  PL : mi' = mu' * invv
  DVE: P fused STT (+acc), Sx TS (+acc), Sm TS (+acc, chunks 1-2)

Output: acc [128, 15] f32, DMA'd out as two partition-halves on the two
HWDGE queues in parallel; the host combines all cores in float64.
"""

import sys

sys.path.insert(0, "/opt/trn_rl_repo")

import numpy as np
from contextlib import ExitStack

import concourse.bass as bass
import concourse.bacc as bacc
import concourse.tile as tile
from concourse import mybir
from concourse.bass_utils import run_bass_kernel_spmd

F32 = mybir.dt.float32
N_CORES = 8
B, D, H, W = 16, 64, 32, 32
HW = H * W                # 1024
N = B * HW                # 16384
NB = B // N_CORES         # 2 sub-slabs (batches) per core
ROWS = NB * HW            # 2048 rows per core
COLS = HW                 # free size of the (128, 1024) layout
QUANT = ["P", "Sx", "Sm", "Sx2", "Sinvv"]
BOUNDS = [0, 448, 832, 1024]
NCH = len(BOUNDS) - 1
CS = [BOUNDS[h + 1] - BOUNDS[h] for h in range(NCH)]
# which chunks' Sm pass runs on ACT (copy+acc) vs DVE (TS+acc)
SM_ON_ACT = set()


def build_nc() -> bass.Bass:
    nc = bacc.Bacc()
    ins = [
        nc.dram_tensor(f"in{h}", [128, 3 * CS[h]], F32, kind="ExternalInput")
        for h in range(NCH)
    ]
    accs = nc.dram_tensor("accs", [128, len(QUANT) * NCH], F32,
                          kind="ExternalOutput")

    with ExitStack() as ctx:
        tc = ctx.enter_context(tile.TileContext(nc))
        big = ctx.enter_context(tc.tile_pool(name="big", bufs=1))
        jp = ctx.enter_context(tc.tile_pool(name="jp", bufs=2))
        accp = ctx.enter_context(tc.tile_pool(name="accp", bufs=1))

        # per-chunk arena: [lv | mu' | x | x2 | mi' | invv]
        ar = [big.tile([128, 6 * CS[h]], F32, name=f"ar{h}")
              for h in range(NCH)]
        acc = accp.tile([128, len(QUANT) * NCH], F32)

        def col(q, c):
            i = QUANT.index(q) * NCH + c
            return acc[:, i:i + 1]

        # The act-table load (1.3us) runs at ACT engine start and serializes
        # ahead of any ACT-issued descriptor-gen, so the ACT queue can't
        # stream before ~4us.  Give SP the first and last chunks and ACT the
        # middle one; SP's gens go back-to-back at the top of its program.
        qs = {0: nc.sync, 1: nc.scalar, 2: nc.sync}
        for h in (0, 2, 1):
            C = CS[h]
            qs[h].dma_start(out=ar[h][:, 0:3 * C], in_=ins[h][:, :])

        M = mybir.AluOpType.mult
        ADD = mybir.AluOpType.add
        EXP = mybir.ActivationFunctionType.Exp
        SQ = mybir.ActivationFunctionType.Square
        CP = mybir.ActivationFunctionType.Copy

        for h in range(NCH):
            C = CS[h]
            lv_s = ar[h][:, 0:C]
            mu_s = ar[h][:, C:2 * C]
            x_s = ar[h][:, 2 * C:3 * C]
            x2_s = ar[h][:, 3 * C:4 * C]
            mi_s = ar[h][:, 4 * C:5 * C]
            iv_s = ar[h][:, 5 * C:6 * C]
            xx2_s = ar[h][:, 2 * C:4 * C]   # [x | x2]
            miiv_s = ar[h][:, 4 * C:6 * C]  # [mi' | invv]

            nc.scalar.activation(
                out=iv_s, in_=lv_s, func=EXP, bias=0.0, scale=-1.0,
                accum_out=col("Sinvv", h),
            )
            nc.scalar.activation(
                out=x2_s, in_=x_s, func=SQ, bias=0.0, scale=1.0,
                accum_out=col("Sx2", h),
            )
            nc.gpsimd.tensor_tensor(mi_s, mu_s, iv_s, op=M)

            # P = sum x*mi' + x2*invv = A - 2B, one fused pass
            jp_t = jp.tile([128, 2 * C], F32, tag="jp", name=f"jp{h}")
            nc.vector.scalar_tensor_tensor(
                out=jp_t[:], in0=xx2_s, scalar=1.0, in1=miiv_s,
                op0=M, op1=M, accum_out=col("P", h),
            )
            jx = jp.tile([128, C], F32, tag="jx", name=f"jx{h}")
            nc.vector.tensor_scalar(
                out=jx[:], in0=x_s, scalar1=1.0, scalar2=0.0,
                op0=M, op1=ADD, accum_out=col("Sx", h),
            )
            if h in SM_ON_ACT:
                jm = jp.tile([128, C], F32, tag="jm", name=f"jm{h}")
                nc.scalar.activation(
                    out=jm[:], in_=mi_s, func=CP, bias=0.0, scale=1.0,
                    accum_out=col("Sm", h),
                )
            else:
                jm = jp.tile([128, C], F32, tag="jm", name=f"jm{h}")
                nc.vector.tensor_scalar(
                    out=jm[:], in0=mi_s, scalar1=1.0, scalar2=0.0,
                    op0=M, op1=ADD, accum_out=col("Sm", h),
                )

        nc.sync.dma_start(out=accs[0:64, :], in_=acc[0:64, :],
                          single_packet=True)
        nc.scalar.dma_start(out=accs[64:128, :], in_=acc[64:128, :],
                            single_packet=True)
    return nc


def _ensure_ntff_hook():
    """This image's antenv lacks axon_hooks; if tracing is requested
    (e.g. BASS_TRACE=1), run_bass_kernel_spmd would die on the import.
    Register the ctypes-based hook if available, else a None hook so
    tracing degrades gracefully."""
    import types

    if "antenv.axon_hooks" in sys.modules:
        return
    try:
        import antenv.axon_hooks  # noqa: F401
        return
    except ImportError:
        pass
    hook = None
    try:
        sys.path.insert(0, "/root/.axon_site")
        from trn_agent_boot.trn_boot import _ntff_profile_via_ctypes

        hook = _ntff_profile_via_ctypes("/opt/axon/libaxon_pjrt.so")
    except Exception:
        hook = None
    mod = types.ModuleType("antenv.axon_hooks")
    mod._hook = hook
    mod.get_axon_ntff_profile_hook = lambda: mod._hook
    mod.set_axon_ntff_profile_hook = lambda h: setattr(mod, "_hook", h)
    sys.modules["antenv.axon_hooks"] = mod


_ensure_ntff_hook()

_NC = None


def _get_nc():
    global _NC
    if _NC is None:
        _NC = build_nc()
        # bacc passes legalize multi-sync-wait instructions for TRN2 codegen
        _NC.compile()
    return _NC


def make_in_maps(x, mu, logvar):
    x = np.ascontiguousarray(np.asarray(x, dtype=np.float32))
    mu = np.asarray(mu, dtype=np.float32)
    lv = np.asarray(logvar, dtype=np.float32)
    in_maps = []
    for c in range(N_CORES):
        r0 = c * ROWS
        mu_t = np.concatenate(
            [mu[r0 + b * HW:r0 + (b + 1) * HW].T for b in range(NB)], axis=0
        ) * np.float32(-2.0)
        lv_t = np.concatenate(
            [lv[r0 + b * HW:r0 + (b + 1) * HW].T for b in range(NB)], axis=0
        )
        x_t = x[c * NB:(c + 1) * NB].reshape(128, COLS)
        m = {}
        for h in range(NCH):
            sl = slice(BOUNDS[h], BOUNDS[h + 1])
            m[f"in{h}"] = np.ascontiguousarray(
                np.concatenate([lv_t[:, sl], mu_t[:, sl], x_t[:, sl]], axis=1)
            )
        in_maps.append(m)
    return in_maps


def combine(results) -> np.ndarray:
    nq = len(QUANT)
    tot = np.zeros((nq, 128), dtype=np.float64)
    for r in results:
        a = np.asarray(r["accs"], dtype=np.float64)  # (128, nq*NCH)
        for q in range(nq):
            tot[q] += a[:, q * NCH:(q + 1) * NCH].sum(axis=1)
    P = tot[QUANT.index("P")].sum()
    vec = {q: tot[QUANT.index(q)].reshape(NB, D).sum(axis=0)
           for q in ("Sx", "Sm", "Sx2", "Sinvv")}
    # Sm is -2*S_muinvv, so  S_invv.S_x2 - 2*S_muinvv.S_x  =  Si.Sx2 + Sm.Sx
    loss = (-0.5 / N * P
            + 0.5 / float(N) ** 2 * (vec["Sinvv"] @ vec["Sx2"]
                                     + vec["Sm"] @ vec["Sx"]))
    return np.array(loss, dtype=np.float32)


def kernel(x, mu, logvar, **_kwargs):
    nc = _get_nc()
    in_maps = make_in_maps(x, mu, logvar)
    res = run_bass_kernel_spmd(nc, in_maps, list(range(N_CORES)))
    return combine(res.results)


# revision 15
# speedup vs baseline: 1.0117x; 1.0044x over previous
"""CLUB loss kernel for Trainium2 (8 NeuronCores, SPMD row-sharded).

Math: the reference returns mean_i(pos_i - neg_i), a scalar:

  mean_pos = -0.5/N * (A - 2B + C)
      A = sum_{i,d} x^2 * invv,  B = sum x*mu*invv,  C = sum mu^2*invv
  mean_neg = -0.5/N^2 * (S_invv . S_x2 - 2*S_muinvv . S_x + N*C)
  loss = mean_pos - mean_neg

C cancels exactly in the loss, so we never compute it.  The host also
pre-scales mu' = -2*mu, which folds the -2B into a single fused sum:

  P := A - 2B = sum([x | x^2] * [mu'*invv | invv])   (one STT pass)
  loss = -0.5/N * P + 0.5/N^2 * (S_invv.S_x2 + S_mi'.S_x)
      where S_mi' = sum mu'*invv = -2*S_muinvv

Each core handles 2048 rows; layout is d-major (128, 1024): partition
q = (sub-slab b, dim d), free axis = row index, so every reduction is a
free-axis row-sum fused into the producing instruction via accum_out.

Per column-chunk h the SBUF arena tile is laid out
  [ lv | mu' | x | x2 | mi' | invv ]   (6*C cols)
so the DMA lands [lv|mu'|x] as ONE contiguous dma_start (128 descriptors
of 3*C*4 bytes - large packets keep the HWDGE queues at full rate), and
the P pass reads the adjacent [x|x2] and [mi'|invv] spans as single APs.

Compute per chunk (engines balanced; walrus rejects gpsimd STT/TS and
accum, so PL only does its TensorTensor product):
  ACT: invv = exp(-lv) (+Sinvv), x2 = x^2 (+Sx2), Sm copy-acc (chunk 0)
  PL : mi' = mu' * invv
  DVE: P fused STT (+acc), Sx TS (+acc), Sm TS (+acc, chunks 1-2)

Output: acc [128, 15] f32, DMA'd out as two partition-halves on the two
HWDGE queues in parallel; the host combines all cores in float64.
"""

import sys

sys.path.insert(0, "/opt/trn_rl_repo")

import numpy as np
from contextlib import ExitStack

import concourse.bass as bass
import concourse.bacc as bacc
import concourse.tile as tile
from concourse import mybir
from concourse.bass_utils import run_bass_kernel_spmd

F32 = mybir.dt.float32
N_CORES = 8
B, D, H, W = 16, 64, 32, 32
HW = H * W                # 1024
N = B * HW                # 16384
NB = B // N_CORES         # 2 sub-slabs (batches) per core
ROWS = NB * HW            # 2048 rows per core
COLS = HW                 # free size of the (128, 1024) layout
QUANT = ["P", "Sx", "Sm", "Sx2", "Sinvv"]
BOUNDS = [0, 384, 704, 1024]
NCH = len(BOUNDS) - 1
CS = [BOUNDS[h + 1] - BOUNDS[h] for h in range(NCH)]
# which chunks' Sm pass runs on ACT (copy+acc) vs DVE (TS+acc)
SM_ON_ACT = set()


def build_nc() -> bass.Bass:
    nc = bacc.Bacc()
    ins = [
        nc.dram_tensor(f"in{h}", [128, 3 * CS[h]], F32, kind="ExternalInput")
        for h in range(NCH)
    ]
    # acc partials split into chunks {0,1} and chunk {2} so the first
    # output DMA can launch while chunk 2 is still computing
    accsA = nc.dram_tensor("accsA", [128, len(QUANT) * 2], F32,
                           kind="ExternalOutput")
    accsB = nc.dram_tensor("accsB", [128, len(QUANT)], F32,
                           kind="ExternalOutput")

    with ExitStack() as ctx:
        tc = ctx.enter_context(tile.TileContext(nc))
        big = ctx.enter_context(tc.tile_pool(name="big", bufs=1))
        jp = ctx.enter_context(tc.tile_pool(name="jp", bufs=2))
        accp = ctx.enter_context(tc.tile_pool(name="accp", bufs=1))

        # per-chunk arena: [lv | mu' | x | x2 | mi' | invv]
        ar = [big.tile([128, 6 * CS[h]], F32, name=f"ar{h}")
              for h in range(NCH)]
        accA = accp.tile([128, len(QUANT) * 2], F32, name="accA")
        accB = accp.tile([128, len(QUANT)], F32, name="accB")

        def col(q, c):
            if c < 2:
                i = QUANT.index(q) * 2 + c
                return accA[:, i:i + 1]
            i = QUANT.index(q)
            return accB[:, i:i + 1]

        # The act-table load (1.3us) runs at ACT engine start and serializes
        # ahead of any ACT-issued descriptor-gen, so the ACT queue can't
        # stream before ~4us.  Give SP the first and last chunks and ACT the
        # middle one; SP's gens go back-to-back at the top of its program.
        qs = {0: nc.sync, 1: nc.scalar, 2: nc.sync}
        for h in (0, 2, 1):
            C = CS[h]
            qs[h].dma_start(out=ar[h][:, 0:3 * C], in_=ins[h][:, :])

        M = mybir.AluOpType.mult
        ADD = mybir.AluOpType.add
        EXP = mybir.ActivationFunctionType.Exp
        SQ = mybir.ActivationFunctionType.Square
        CP = mybir.ActivationFunctionType.Copy

        for h in range(NCH):
            # Pin the scheduler's per-engine order to physical chunk-arrival
            # order: its DMA model doesn't know the act-table load delays the
            # ACT queue, so left alone it reorders (e.g. exp1 before exp0)
            # and idles engines against the slowest queue.
            tc.tile_set_cur_wait(0.001 * (h + 1))
            C = CS[h]
            lv_s = ar[h][:, 0:C]
            mu_s = ar[h][:, C:2 * C]
            x_s = ar[h][:, 2 * C:3 * C]
            x2_s = ar[h][:, 3 * C:4 * C]
            mi_s = ar[h][:, 4 * C:5 * C]
            iv_s = ar[h][:, 5 * C:6 * C]
            xx2_s = ar[h][:, 2 * C:4 * C]   # [x | x2]
            miiv_s = ar[h][:, 4 * C:6 * C]  # [mi' | invv]

            nc.scalar.activation(
                out=iv_s, in_=lv_s, func=EXP, bias=0.0, scale=-1.0,
                accum_out=col("Sinvv", h),
            )
            nc.scalar.activation(
                out=x2_s, in_=x_s, func=SQ, bias=0.0, scale=1.0,
                accum_out=col("Sx2", h),
            )
            nc.gpsimd.tensor_tensor(mi_s, mu_s, iv_s, op=M)

            # P = sum x*mi' + x2*invv = A - 2B, one fused pass
            jp_t = jp.tile([128, 2 * C], F32, tag="jp", name=f"jp{h}")
            nc.vector.scalar_tensor_tensor(
                out=jp_t[:], in0=xx2_s, scalar=1.0, in1=miiv_s,
                op0=M, op1=M, accum_out=col("P", h),
            )
            jx = jp.tile([128, C], F32, tag="jx", name=f"jx{h}")
            nc.vector.tensor_scalar(
                out=jx[:], in0=x_s, scalar1=1.0, scalar2=0.0,
                op0=M, op1=ADD, accum_out=col("Sx", h),
            )
            if h in SM_ON_ACT:
                jm = jp.tile([128, C], F32, tag="jm", name=f"jm{h}")
                nc.scalar.activation(
                    out=jm[:], in_=mi_s, func=CP, bias=0.0, scale=1.0,
                    accum_out=col("Sm", h),
                )
            else:
                jm = jp.tile([128, C], F32, tag="jm", name=f"jm{h}")
                nc.vector.tensor_scalar(
                    out=jm[:], in0=mi_s, scalar1=1.0, scalar2=0.0,
                    op0=M, op1=ADD, accum_out=col("Sm", h),
                )

        # accA (chunks 0-1) goes out while chunk 2 still computes; accB's
        # two partition-halves go out in parallel at the end.
        tc.tile_set_cur_wait(0.01)
        nc.sync.dma_start(out=accsA[:, :], in_=accA[:])
        tc.tile_set_cur_wait(0.02)
        nc.sync.dma_start(out=accsB[0:64, :], in_=accB[0:64, :])
        nc.scalar.dma_start(out=accsB[64:128, :], in_=accB[64:128, :])
    return nc


def _ensure_ntff_hook():
    """This image's antenv lacks axon_hooks; if tracing is requested
    (e.g. BASS_TRACE=1), run_bass_kernel_spmd would die on the import.
    Register the ctypes-based hook if available, else a None hook so
    tracing degrades gracefully."""
    import types

    if "antenv.axon_hooks" in sys.modules:
        return
    try:
        import antenv.axon_hooks  # noqa: F401
        return
    except ImportError:
        pass
    hook = None
    try:
        sys.path.insert(0, "/root/.axon_site")
        from trn_agent_boot.trn_boot import _ntff_profile_via_ctypes

        hook = _ntff_profile_via_ctypes("/opt/axon/libaxon_pjrt.so")
    except Exception:
        hook = None
    mod = types.ModuleType("antenv.axon_hooks")
    mod._hook = hook
    mod.get_axon_ntff_profile_hook = lambda: mod._hook
    mod.set_axon_ntff_profile_hook = lambda h: setattr(mod, "_hook", h)
    sys.modules["antenv.axon_hooks"] = mod


_ensure_ntff_hook()

_NC = None


def _get_nc():
    global _NC
    if _NC is None:
        _NC = build_nc()
        # bacc passes legalize multi-sync-wait instructions for TRN2 codegen
        _NC.compile()
    return _NC


def make_in_maps(x, mu, logvar):
    x = np.ascontiguousarray(np.asarray(x, dtype=np.float32))
    mu = np.asarray(mu, dtype=np.float32)
    lv = np.asarray(logvar, dtype=np.float32)
    in_maps = []
    for c in range(N_CORES):
        r0 = c * ROWS
        mu_t = np.concatenate(
            [mu[r0 + b * HW:r0 + (b + 1) * HW].T for b in range(NB)], axis=0
        ) * np.float32(-2.0)
        lv_t = np.concatenate(
            [lv[r0 + b * HW:r0 + (b + 1) * HW].T for b in range(NB)], axis=0
        )
        x_t = x[c * NB:(c + 1) * NB].reshape(128, COLS)
        m = {}
        for h in range(NCH):
            sl = slice(BOUNDS[h], BOUNDS[h + 1])
            m[f"in{h}"] = np.ascontiguousarray(
                np.concatenate([lv_t[:, sl], mu_t[:, sl], x_t[:, sl]], axis=1)
            )
        in_maps.append(m)
    return in_maps


def combine(results) -> np.ndarray:
    nq = len(QUANT)
    tot = np.zeros((nq, 128), dtype=np.float64)
    for r in results:
        a = np.asarray(r["accsA"], dtype=np.float64)  # (128, nq*2)
        b = np.asarray(r["accsB"], dtype=np.float64)  # (128, nq)
        for q in range(nq):
            tot[q] += a[:, q * 2:(q + 1) * 2].sum(axis=1) + b[:, q]
    P = tot[QUANT.index("P")].sum()
    vec = {q: tot[QUANT.index(q)].reshape(NB, D).sum(axis=0)
           for q in ("Sx", "Sm", "Sx2", "Sinvv")}
    # Sm is -2*S_muinvv, so  S_invv.S_x2 - 2*S_muinvv.S_x  =  Si.Sx2 + Sm.Sx
    loss = (-0.5 / N * P
            + 0.5 / float(N) ** 2 * (vec["Sinvv"] @ vec["Sx2"]
                                     + vec["Sm"] @ vec["Sx"]))
    return np.array(loss, dtype=np.float32)


def kernel(x, mu, logvar, **_kwargs):
    nc = _get_nc()
    in_maps = make_in_maps(x, mu, logvar)
    res = run_bass_kernel_spmd(nc, in_maps, list(range(N_CORES)))
    return combine(res.results)


# revision 18
# speedup vs baseline: 1.0307x; 1.0188x over previous
"""CLUB loss kernel for Trainium2 (8 NeuronCores, SPMD row-sharded).

Math: the reference returns mean_i(pos_i - neg_i), a scalar:

  mean_pos = -0.5/N * (A - 2B + C)
      A = sum_{i,d} x^2 * invv,  B = sum x*mu*invv,  C = sum mu^2*invv
  mean_neg = -0.5/N^2 * (S_invv . S_x2 - 2*S_muinvv . S_x + N*C)
  loss = mean_pos - mean_neg

C cancels exactly in the loss, so we never compute it.  The host also
pre-scales mu' = -2*mu, which folds the -2B into a single fused sum:

  P := A - 2B = sum([x | x^2] * [mu'*invv | invv])   (one STT pass)
  loss = -0.5/N * P + 0.5/N^2 * (S_invv.S_x2 + S_mi'.S_x)
      where S_mi' = sum mu'*invv = -2*S_muinvv

Each core handles 2048 rows; layout is d-major (128, 1024): partition
q = (sub-slab b, dim d), free axis = row index, so every reduction is a
free-axis row-sum fused into the producing instruction via accum_out.

The three 352-column chunks (the last zero-padded from 320) live in ONE
SBUF arena tile; chunk h occupies block h*2112 laid out as
  [ lv | mu' | x | x2 | mi' | invv ]   (352 cols each)
so (a) each chunk's DMA is ONE contiguous dma_start (128 descriptors of
4224B - large packets keep the HWDGE queues at full rate), (b) the P
pass reads the adjacent [x|x2] / [mi'|invv] spans as single APs, and
(c) Sx and Sm collapse to ONE strided-AP pass each over all chunks
([[2112,3],[1,352]]), halving the standalone-reduction work on DVE.
Zero pad => pad contributes 0 to every sum except Sinvv (exp(0)=1),
corrected by -32/partition on the host.

Chunk sizes are EQUAL on purpose: the tile scheduler orders each
engine's stream by its modeled DMA completion times (it does not know
the act-table load delays the ACT queue by ~2.5us), and with equal
sizes it falls back to emission order, which matches physical arrival
(SP: ch0 first, ACT: ch1, SP: ch2).

Engines per chunk: ACT: exp (+Sinvv ride), x^2 (+Sx2 ride); PL: mi'
(plain TensorTensor - walrus rejects everything else on gpsimd);
DVE: fused P.  Then DVE: Sx-all, Sm-all.  Partials split into accA
(chunks 0-1, DMA'd out while chunk 2 computes) and accB (end).
"""

import sys

sys.path.insert(0, "/opt/trn_rl_repo")

import numpy as np
from contextlib import ExitStack

import concourse.bass as bass
import concourse.bacc as bacc
import concourse.tile as tile
from concourse import mybir
from concourse.bass_utils import run_bass_kernel_spmd

F32 = mybir.dt.float32
N_CORES = 8
B, D, H, W = 16, 64, 32, 32
HW = H * W                # 1024
N = B * HW                # 16384
NB = B // N_CORES         # 2 sub-slabs (batches) per core
ROWS = NB * HW            # 2048 rows per core
COLS = HW                 # real cols of the (128, 1024) layout
C = 352                   # chunk cols (last chunk: 320 real + 32 zero pad)
NCH = 3
PAD = NCH * C - COLS      # 32
BLK = 6 * C               # arena block stride per chunk
# acc column maps
A_COLS = {("P", 0): 0, ("P", 1): 1, ("Sinvv", 0): 2, ("Sinvv", 1): 3,
          ("Sx2", 0): 4, ("Sx2", 1): 5}
B_COLS = {("P", 2): 0, ("Sinvv", 2): 1, ("Sx2", 2): 2,
          ("Sx", None): 3, ("Sm", None): 4}


def build_nc() -> bass.Bass:
    nc = bacc.Bacc()
    ins = [
        nc.dram_tensor(f"in{h}", [128, 3 * C], F32, kind="ExternalInput")
        for h in range(NCH)
    ]
    accsA = nc.dram_tensor("accsA", [128, len(A_COLS)], F32,
                           kind="ExternalOutput")
    accsB = nc.dram_tensor("accsB", [128, len(B_COLS)], F32,
                           kind="ExternalOutput")

    with ExitStack() as ctx:
        tc = ctx.enter_context(tile.TileContext(nc))
        big = ctx.enter_context(tc.tile_pool(name="big", bufs=1))
        jp = ctx.enter_context(tc.tile_pool(name="jp", bufs=2))
        accp = ctx.enter_context(tc.tile_pool(name="accp", bufs=1))

        ar = big.tile([128, NCH * BLK], F32, name="arena")
        accA = accp.tile([128, len(A_COLS)], F32, name="accA")
        accB = accp.tile([128, len(B_COLS)], F32, name="accB")

        def col(q, c):
            if (q, c) in A_COLS:
                i = A_COLS[(q, c)]
                return accA[:, i:i + 1]
            i = B_COLS[(q, c)]
            return accB[:, i:i + 1]

        qs = {0: nc.sync, 1: nc.scalar, 2: nc.sync}
        for h in (0, 2, 1):
            qs[h].dma_start(out=ar[:, h * BLK:h * BLK + 3 * C],
                            in_=ins[h][:, :])

        M = mybir.AluOpType.mult
        ADD = mybir.AluOpType.add
        EXP = mybir.ActivationFunctionType.Exp
        SQ = mybir.ActivationFunctionType.Square

        for h in range(NCH):
            o = h * BLK
            lv_s = ar[:, o:o + C]
            mu_s = ar[:, o + C:o + 2 * C]
            x_s = ar[:, o + 2 * C:o + 3 * C]
            x2_s = ar[:, o + 3 * C:o + 4 * C]
            mi_s = ar[:, o + 4 * C:o + 5 * C]
            iv_s = ar[:, o + 5 * C:o + 6 * C]
            xx2_s = ar[:, o + 2 * C:o + 4 * C]   # [x | x2]
            miiv_s = ar[:, o + 4 * C:o + 6 * C]  # [mi' | invv]

            nc.scalar.activation(
                out=iv_s, in_=lv_s, func=EXP, bias=0.0, scale=-1.0,
                accum_out=col("Sinvv", h),
            )
            nc.scalar.activation(
                out=x2_s, in_=x_s, func=SQ, bias=0.0, scale=1.0,
                accum_out=col("Sx2", h),
            )
            nc.gpsimd.tensor_tensor(mi_s, mu_s, iv_s, op=M)

            jp_t = jp.tile([128, 2 * C], F32, tag="jp", name=f"jp{h}")
            nc.vector.scalar_tensor_tensor(
                out=jp_t[:], in0=xx2_s, scalar=1.0, in1=miiv_s,
                op0=M, op1=M, accum_out=col("P", h),
            )

        # all-chunk strided sums: x runs at block offset 2C, mi' runs at 4C
        ar3 = ar[:, :].rearrange("p (c b) -> p c b", c=NCH)
        jx = jp.tile([128, NCH * C], F32, tag="jx", name="jx")
        jx3 = jx[:].rearrange("p (c b) -> p c b", c=NCH)
        nc.vector.tensor_scalar(
            out=jx3, in0=ar3[:, :, 2 * C:3 * C],
            scalar1=1.0, scalar2=0.0, op0=M, op1=ADD,
            accum_out=col("Sx", None),
        )
        jm = jp.tile([128, NCH * C], F32, tag="jm", name="jm")
        jm3 = jm[:].rearrange("p (c b) -> p c b", c=NCH)
        nc.vector.tensor_scalar(
            out=jm3, in0=ar3[:, :, 4 * C:5 * C],
            scalar1=1.0, scalar2=0.0, op0=M, op1=ADD,
            accum_out=col("Sm", None),
        )

        # accA (chunks 0-1) leaves while chunk 2 computes; accB at the end
        nc.sync.dma_start(out=accsA[:, :], in_=accA[:])
        nc.sync.dma_start(out=accsB[0:64, :], in_=accB[0:64, :])
        nc.scalar.dma_start(out=accsB[64:128, :], in_=accB[64:128, :])
    return nc


def _ensure_ntff_hook():
    """This image's antenv lacks axon_hooks; if tracing is requested
    (e.g. BASS_TRACE=1), run_bass_kernel_spmd would die on the import.
    Register the ctypes-based hook if available, else a None hook so
    tracing degrades gracefully."""
    import types

    if "antenv.axon_hooks" in sys.modules:
        return
    try:
        import antenv.axon_hooks  # noqa: F401
        return
    except ImportError:
        pass
    hook = None
    try:
        sys.path.insert(0, "/root/.axon_site")
        from trn_agent_boot.trn_boot import _ntff_profile_via_ctypes

        hook = _ntff_profile_via_ctypes("/opt/axon/libaxon_pjrt.so")
    except Exception:
        hook = None
    mod = types.ModuleType("antenv.axon_hooks")
    mod._hook = hook
    mod.get_axon_ntff_profile_hook = lambda: mod._hook
    mod.set_axon_ntff_profile_hook = lambda h: setattr(mod, "_hook", h)
    sys.modules["antenv.axon_hooks"] = mod


_ensure_ntff_hook()

_NC = None


def _get_nc():
    global _NC
    if _NC is None:
        _NC = build_nc()
        # bacc passes legalize multi-sync-wait instructions for TRN2 codegen
        _NC.compile()
    return _NC


def make_in_maps(x, mu, logvar):
    x = np.ascontiguousarray(np.asarray(x, dtype=np.float32))
    mu = np.asarray(mu, dtype=np.float32)
    lv = np.asarray(logvar, dtype=np.float32)
    in_maps = []
    for c in range(N_CORES):
        r0 = c * ROWS
        mu_t = np.concatenate(
            [mu[r0 + b * HW:r0 + (b + 1) * HW].T for b in range(NB)], axis=0
        ) * np.float32(-2.0)
        lv_t = np.concatenate(
            [lv[r0 + b * HW:r0 + (b + 1) * HW].T for b in range(NB)], axis=0
        )
        x_t = x[c * NB:(c + 1) * NB].reshape(128, COLS)
        pad = np.zeros((128, PAD), dtype=np.float32)
        mu_t = np.concatenate([mu_t, pad], axis=1)
        lv_t = np.concatenate([lv_t, pad], axis=1)
        x_t = np.concatenate([x_t, pad], axis=1)
        m = {}
        for h in range(NCH):
            sl = slice(h * C, (h + 1) * C)
            m[f"in{h}"] = np.ascontiguousarray(
                np.concatenate([lv_t[:, sl], mu_t[:, sl], x_t[:, sl]], axis=1)
            )
        in_maps.append(m)
    return in_maps


def combine(results) -> np.ndarray:
    P = 0.0
    vec = {q: np.zeros(128, dtype=np.float64) for q in
           ("Sx", "Sm", "Sx2", "Sinvv")}
    for r in results:
        a = np.asarray(r["accsA"], dtype=np.float64)
        b = np.asarray(r["accsB"], dtype=np.float64)
        P += a[:, 0].sum() + a[:, 1].sum() + b[:, 0].sum()
        # pad cols contribute exp(-0) = 1 each to the chunk-2 Sinvv partial
        vec["Sinvv"] += a[:, 2] + a[:, 3] + (b[:, 1] - float(PAD))
        vec["Sx2"] += a[:, 4] + a[:, 5] + b[:, 2]
        vec["Sx"] += b[:, 3]
        vec["Sm"] += b[:, 4]
    v = {q: vec[q].reshape(NB, D).sum(axis=0) for q in vec}
    # Sm is -2*S_muinvv, so  S_invv.S_x2 - 2*S_muinvv.S_x  =  Si.Sx2 + Sm.Sx
    loss = (-0.5 / N * P
            + 0.5 / float(N) ** 2 * (v["Sinvv"] @ v["Sx2"]
                                     + v["Sm"] @ v["Sx"]))
    return np.array(loss, dtype=np.float32)


def kernel(x, mu, logvar, **_kwargs):
    nc = _get_nc()
    in_maps = make_in_maps(x, mu, logvar)
    res = run_bass_kernel_spmd(nc, in_maps, list(range(N_CORES)))
    return combine(res.results)


# revision 19
# speedup vs baseline: 1.0849x; 1.0525x over previous
"""CLUB loss kernel for Trainium2 (8 NeuronCores, SPMD row-sharded).

Math: the reference returns mean_i(pos_i - neg_i), a scalar:

  mean_pos = -0.5/N * (A - 2B + C)
      A = sum_{i,d} x^2 * invv,  B = sum x*mu*invv,  C = sum mu^2*invv
  mean_neg = -0.5/N^2 * (S_invv . S_x2 - 2*S_muinvv . S_x + N*C)
  loss = mean_pos - mean_neg

C cancels exactly in the loss, so we never compute it.  The host also
pre-scales mu' = -2*mu, which folds the -2B into a single fused sum:

  P := A - 2B = sum([x | x^2] * [mu'*invv | invv])   (one STT pass)
  loss = -0.5/N * P + 0.5/N^2 * (S_invv.S_x2 + S_mi'.S_x)
      where S_mi' = sum mu'*invv = -2*S_muinvv

Each core handles 2048 rows; layout is d-major (128, 1024): partition
q = (sub-slab b, dim d), free axis = row index, so every reduction is a
free-axis row-sum fused into the producing instruction via accum_out.

Chunks of [256, 384, 384] columns live in ONE SBUF arena tile; chunk h
is a block laid out as [ lv | mu' | x | x2 | mi' | invv ] so (a) each
chunk's DMA is ONE contiguous dma_start (128 descriptors of 3*C*4 bytes
- large packets keep the HWDGE queues near their ~250GB/s cap), (b) the
P pass reads the adjacent [x|x2] / [mi'|invv] spans as single 2-level
APs, and (c) the chunk-1+2 sums collapse into single strided-AP passes.

Queue/size choices keep the tile scheduler honest: it orders each
engine's stream by MODELED DMA completion (it does not know the
act-table load delays the ACT queue by ~2.5us), so per-queue cumulative
sizes must be increasing in emission order (SP:256 | ACT:384 | SP:640).

Engines: ACT: exp (+Sinvv), x^2 (+Sx2) per chunk, plus the chunk-0
Sx/Sm copy-accs in its chunk-0/1 gap and the Sm-pair at the end;
PL: mi' per chunk (plain TensorTensor - walrus rejects all other
gpsimd compute and any gpsimd accum_out); DVE: fused P per chunk and
the strided Sx-pair.  Partials: accA (chunks 0-1) is DMA'd out while
chunk 2 computes; accB goes out at the end on the software-DGE (PL)
queue, skipping the HWDGE descriptor-expansion latency.
"""

import sys

sys.path.insert(0, "/opt/trn_rl_repo")

import numpy as np
from contextlib import ExitStack

import concourse.bass as bass
import concourse.bacc as bacc
import concourse.tile as tile
from concourse import mybir
from concourse.bass_utils import run_bass_kernel_spmd

F32 = mybir.dt.float32
N_CORES = 8
B, D, H, W = 16, 64, 32, 32
HW = H * W                # 1024
N = B * HW                # 16384
NB = B // N_CORES         # 2 sub-slabs (batches) per core
ROWS = NB * HW            # 2048 rows per core
COLS = HW                 # 1024 free cols in the (128, 1024) layout
CS = [256, 384, 384]      # chunk cols (sum == COLS, ch1 == ch2 for pairing)
NCH = 3
OFF = [0, 6 * CS[0], 6 * (CS[0] + CS[1])]   # arena block offsets
A_COLS = {("P", 0): 0, ("P", 1): 1, ("Sinvv", 0): 2, ("Sinvv", 1): 3,
          ("Sx2", 0): 4, ("Sx2", 1): 5, ("Sx", 0): 6, ("Sm", 0): 7}
B_COLS = {("P", 2): 0, ("Sinvv", 2): 1, ("Sx2", 2): 2,
          ("Sx", 12): 3, ("Sm", 12): 4}


def build_nc() -> bass.Bass:
    nc = bacc.Bacc()
    ins = [
        nc.dram_tensor(f"in{h}", [128, 3 * CS[h]], F32, kind="ExternalInput")
        for h in range(NCH)
    ]
    accsA = nc.dram_tensor("accsA", [128, len(A_COLS)], F32,
                           kind="ExternalOutput")
    accsB = nc.dram_tensor("accsB", [128, len(B_COLS)], F32,
                           kind="ExternalOutput")

    with ExitStack() as ctx:
        tc = ctx.enter_context(tile.TileContext(nc))
        big = ctx.enter_context(tc.tile_pool(name="big", bufs=1))
        jp = ctx.enter_context(tc.tile_pool(name="jp", bufs=2))
        accp = ctx.enter_context(tc.tile_pool(name="accp", bufs=1))

        ar = big.tile([128, 6 * COLS], F32, name="arena")
        accA = accp.tile([128, len(A_COLS)], F32, name="accA")
        accB = accp.tile([128, len(B_COLS)], F32, name="accB")

        def col(q, c):
            if (q, c) in A_COLS:
                i = A_COLS[(q, c)]
                return accA[:, i:i + 1]
            i = B_COLS[(q, c)]
            return accB[:, i:i + 1]

        qs = {0: nc.sync, 1: nc.scalar, 2: nc.sync}
        for h in (0, 2, 1):
            qs[h].dma_start(out=ar[:, OFF[h]:OFF[h] + 3 * CS[h]],
                            in_=ins[h][:, :])

        M = mybir.AluOpType.mult
        ADD = mybir.AluOpType.add
        EXP = mybir.ActivationFunctionType.Exp
        SQ = mybir.ActivationFunctionType.Square
        CP = mybir.ActivationFunctionType.Copy

        def blk(h, j0, j1):
            C = CS[h]
            return ar[:, OFF[h] + j0 * C:OFF[h] + j1 * C]

        for h in range(NCH):
            nc.scalar.activation(
                out=blk(h, 5, 6), in_=blk(h, 0, 1), func=EXP, bias=0.0,
                scale=-1.0, accum_out=col("Sinvv", h),
            )
            nc.scalar.activation(
                out=blk(h, 3, 4), in_=blk(h, 2, 3), func=SQ, bias=0.0,
                scale=1.0, accum_out=col("Sx2", h),
            )
            nc.gpsimd.tensor_tensor(blk(h, 4, 5), blk(h, 1, 2),
                                    blk(h, 5, 6), op=M)
            jp_t = jp.tile([128, 2 * CS[h]], F32, tag="jp", name=f"jp{h}")
            nc.vector.scalar_tensor_tensor(
                out=jp_t[:], in0=blk(h, 2, 4), scalar=1.0, in1=blk(h, 4, 6),
                op0=M, op1=M, accum_out=col("P", h),
            )
            if h == 0:
                # chunk-0 sums ride ACT's gap between chunk 0 and chunk 1
                ja = jp.tile([128, CS[0]], F32, tag="ja", name="sx0")
                nc.scalar.activation(
                    out=ja[:], in_=blk(0, 2, 3), func=CP, bias=0.0, scale=1.0,
                    accum_out=col("Sx", 0),
                )
                jb = jp.tile([128, CS[0]], F32, tag="ja", name="sm0")
                nc.scalar.activation(
                    out=jb[:], in_=blk(0, 4, 5), func=CP, bias=0.0, scale=1.0,
                    accum_out=col("Sm", 0),
                )

        # chunk-1+2 strided pair sums ([[6*384, 2], [1, 384]])
        C = CS[1]
        pair = ar[:, OFF[1]:].rearrange("p (c b) -> p c b", c=2)
        jx = jp.tile([128, 2 * C], F32, tag="jx", name="jx")
        jx2 = jx[:].rearrange("p (c b) -> p c b", c=2)
        nc.vector.tensor_scalar(
            out=jx2, in0=pair[:, :, 2 * C:3 * C],
            scalar1=1.0, scalar2=0.0, op0=M, op1=ADD,
            accum_out=col("Sx", 12),
        )
        jm = jp.tile([128, 2 * C], F32, tag="jm", name="jm")
        jm2 = jm[:].rearrange("p (c b) -> p c b", c=2)
        nc.scalar.activation(
            out=jm2, in_=pair[:, :, 4 * C:5 * C], func=CP, bias=0.0,
            scale=1.0, accum_out=col("Sm", 12),
        )

        # accA leaves while chunk 2 computes; accB at the end via SWDGE
        nc.sync.dma_start(out=accsA[:, :], in_=accA[:])
        nc.gpsimd.dma_start(out=accsB[:, :], in_=accB[:])
    return nc


def _ensure_ntff_hook():
    """This image's antenv lacks axon_hooks; if tracing is requested
    (e.g. BASS_TRACE=1), run_bass_kernel_spmd would die on the import.
    Register the ctypes-based hook if available, else a None hook so
    tracing degrades gracefully."""
    import types

    if "antenv.axon_hooks" in sys.modules:
        return
    try:
        import antenv.axon_hooks  # noqa: F401
        return
    except ImportError:
        pass
    hook = None
    try:
        sys.path.insert(0, "/root/.axon_site")
        from trn_agent_boot.trn_boot import _ntff_profile_via_ctypes

        hook = _ntff_profile_via_ctypes("/opt/axon/libaxon_pjrt.so")
    except Exception:
        hook = None
    mod = types.ModuleType("antenv.axon_hooks")
    mod._hook = hook
    mod.get_axon_ntff_profile_hook = lambda: mod._hook
    mod.set_axon_ntff_profile_hook = lambda h: setattr(mod, "_hook", h)
    sys.modules["antenv.axon_hooks"] = mod


_ensure_ntff_hook()

_NC = None


def _get_nc():
    global _NC
    if _NC is None:
        _NC = build_nc()
        # bacc passes legalize multi-sync-wait instructions for TRN2 codegen
        _NC.compile()
    return _NC


def make_in_maps(x, mu, logvar):
    x = np.ascontiguousarray(np.asarray(x, dtype=np.float32))
    mu = np.asarray(mu, dtype=np.float32)
    lv = np.asarray(logvar, dtype=np.float32)
    in_maps = []
    bounds = np.cumsum([0] + CS)
    for c in range(N_CORES):
        r0 = c * ROWS
        mu_t = np.concatenate(
            [mu[r0 + b * HW:r0 + (b + 1) * HW].T for b in range(NB)], axis=0
        ) * np.float32(-2.0)
        lv_t = np.concatenate(
            [lv[r0 + b * HW:r0 + (b + 1) * HW].T for b in range(NB)], axis=0
        )
        x_t = x[c * NB:(c + 1) * NB].reshape(128, COLS)
        m = {}
        for h in range(NCH):
            sl = slice(bounds[h], bounds[h + 1])
            m[f"in{h}"] = np.ascontiguousarray(
                np.concatenate([lv_t[:, sl], mu_t[:, sl], x_t[:, sl]], axis=1)
            )
        in_maps.append(m)
    return in_maps


def combine(results) -> np.ndarray:
    P = 0.0
    vec = {q: np.zeros(128, dtype=np.float64) for q in
           ("Sx", "Sm", "Sx2", "Sinvv")}
    for r in results:
        a = np.asarray(r["accsA"], dtype=np.float64)
        b = np.asarray(r["accsB"], dtype=np.float64)
        P += a[:, 0].sum() + a[:, 1].sum() + b[:, 0].sum()
        vec["Sinvv"] += a[:, 2] + a[:, 3] + b[:, 1]
        vec["Sx2"] += a[:, 4] + a[:, 5] + b[:, 2]
        vec["Sx"] += a[:, 6] + b[:, 3]
        vec["Sm"] += a[:, 7] + b[:, 4]
    v = {q: vec[q].reshape(NB, D).sum(axis=0) for q in vec}
    # Sm is -2*S_muinvv, so  S_invv.S_x2 - 2*S_muinvv.S_x  =  Si.Sx2 + Sm.Sx
    loss = (-0.5 / N * P
            + 0.5 / float(N) ** 2 * (v["Sinvv"] @ v["Sx2"]
                                     + v["Sm"] @ v["Sx"]))
    return np.array(loss, dtype=np.float32)


def kernel(x, mu, logvar, **_kwargs):
    nc = _get_nc()
    in_maps = make_in_maps(x, mu, logvar)
    res = run_bass_kernel_spmd(nc, in_maps, list(range(N_CORES)))
    return combine(res.results)
